# revision 15
# baseline (speedup 1.0000x reference)
"""Causal self-attention with KV-cache append on 8 Trainium2 NeuronCores.

Sharding: tensor-parallel over heads (4 groups of 4 heads) x data-parallel
over batch (2): core c handles batch b=c//4 and heads [4*(c%4), 4*(c%4)+4).
Each core computes its QKV projections, causal attention for its heads, and a
partial output projection (row-parallel Wo).  The host sums the 4 partial
outputs per batch, adds bo, and assembles the returned KV caches.  Host-side
input marshalling per core: slice the head group, transpose x and the K cache
to channel-major, downcast the matmul-side tensors to bf16, append a ones
column to the V cache (it rides the A.V matmul to produce the softmax
denominator).

Device kernel notes:
 - projections contract over d_model using the host-transposed xT; all
   matmuls run in bf16 (PSUM accumulates fp32); k_new/v_new outputs are
   computed to fp32 with the bias added on the DVE.
 - attention runs in the "scores transposed" layout: scoresT[k, q] tiles of
   [128, 512] = kT.T @ qT, one head at a time, with the score PSUM
   double-buffered against the scalar engine's exp.
 - softmax skips the max subtraction (scores ~ N(0,1) so exp is safe);
   exp runs on the scalar engine straight out of 3-bank PSUM into bf16 sbuf
   tiles; causal masking is gpsimd.affine_select on the exp tiles.
 - normalization: sumexp row (row 64 of the A.V PSUM) -> partition 0 via an
   sbuf->sbuf DMA -> gpsimd partition_broadcast -> DVE reciprocal_approx_fast
   -> multiply; head B's half hops to partitions 64-127 via sbuf->sbuf DMA.
 - the output projection is interleaved per q-tile so the PE never idles at
   the tail.
"""

import sys
import types

import numpy as np
import ml_dtypes

import concourse.bass as bass
import concourse.mybir as mybir
import concourse.tile as tile
from concourse import bacc
from concourse.bass_utils import run_bass_kernel_spmd


def _ensure_axon_ntff_shim():
    """This image's ``antenv`` lacks ``axon_hooks``; provide the tiny
    get/set pair so ``run_bass_kernel_spmd(trace=True)`` works instead of
    crashing on import (it is only touched when tracing is requested)."""
    try:
        import antenv.axon_hooks  # noqa: F401

        return
    except ImportError:
        pass
    try:
        import antenv

        mod = types.ModuleType("antenv.axon_hooks")
        mod._hook = None
        mod.set_axon_ntff_profile_hook = lambda h: setattr(mod, "_hook", h)
        mod.get_axon_ntff_profile_hook = lambda: mod._hook
        sys.modules["antenv.axon_hooks"] = mod
        antenv.axon_hooks = mod
        try:
            from trn_agent_boot.trn_boot import _ntff_profile_via_ctypes

            mod._hook = _ntff_profile_via_ctypes("/opt/axon/libaxon_pjrt.so")
        except Exception:
            pass
    except Exception:
        pass


_ensure_axon_ntff_shim()

# problem shape (hardcoded per harness contract)
B = 2
D = 1024
H = 16
DH = 64
TQ_TOT = 2048
TC = 2048

HPC = H // 4  # heads per core = 4
NP = HPC // 2  # head pairs per core = 2
P = 128
QT = 512  # q tile width
GK = 3  # k-tiles (of 128) per exp group
SCALE = DH**-0.5

F32 = mybir.dt.float32
BF16 = mybir.dt.bfloat16
NPBF = ml_dtypes.bfloat16


def build_nc(tq=TQ_TOT, tc=TC):
    """Build the per-core SPMD program (same program on all 8 cores)."""
    nqt = tq // QT
    nkt_new = tq // P
    nkt_cache = tc // P
    nkt_tot = nkt_new + nkt_cache
    nxt = tq // P  # token tiles
    nch = D // P  # 8 chan chunks
    gw = HPC * DH  # per-core projection width (256)

    nc = bacc.Bacc("TRN2", target_bir_lowering=False, debug=False, num_devices=8)

    xT_d = nc.dram_tensor("xT", [D, tq], BF16, kind="ExternalInput")
    kcpT_d = nc.dram_tensor("kcpT", [NP, P, tc], BF16, kind="ExternalInput")
    vca_d = nc.dram_tensor("vca", [HPC, tc, DH + 2], BF16, kind="ExternalInput")
    wq_d = nc.dram_tensor("wq", [D, gw], BF16, kind="ExternalInput")
    wkv_d = nc.dram_tensor("wkv", [D, 2 * gw], BF16, kind="ExternalInput")
    wo_d = nc.dram_tensor("wo", [gw, D], BF16, kind="ExternalInput")
    bqt_d = nc.dram_tensor("bqt", [P, NP], F32, kind="ExternalInput")
    bkt_d = nc.dram_tensor("bkt", [P, NP], F32, kind="ExternalInput")
    bkvrep_d = nc.dram_tensor("bkvrep", [P, 2 * gw], F32, kind="ExternalInput")

    y_d = nc.dram_tensor("y", [tq, D], F32, kind="ExternalOutput")
    knew_d = nc.dram_tensor("knew", [tq, gw], F32, kind="ExternalOutput")
    vnew_d = nc.dram_tensor("vnew", [tq, gw], F32, kind="ExternalOutput")

    with tile.TileContext(nc) as tc_:
        with (
            tc_.tile_pool(name="const", bufs=1) as const,
            tc_.tile_pool(name="persist", bufs=1) as persist,
            tc_.tile_pool(name="stage", bufs=2) as stage,
            tc_.tile_pool(name="kTcp", bufs=1) as kTcp,
            tc_.tile_pool(name="projw", bufs=1) as projw,
            tc_.tile_pool(name="psD", bufs=2, space="PSUM") as psD,
            tc_.tile_pool(name="avp", bufs=2, space="PSUM") as avp,
            tc_.tile_pool(name="expp", bufs=4) as expp,
            tc_.tile_pool(name="normp", bufs=2) as normp,
        ):
            # ---- constants / inputs straight to sbuf ----
            bqt_sb = const.tile([P, NP], F32, tag="bqt")
            bkt_sb = const.tile([P, NP], F32, tag="bkt")
            nc.gpsimd.dma_start(bqt_sb, bqt_d.ap())
            nc.gpsimd.dma_start(bkt_sb, bkt_d.ap())
            bkvrep_sb = const.tile([P, 2 * gw], F32, tag="bkvrep")
            nc.gpsimd.dma_start(bkvrep_sb, bkvrep_d.ap())
            wo_r = const.tile([P, gw // P, D], BF16, tag="wor")

            # persistent activation tensors
            qT = [
                persist.tile([P, tq], BF16, tag=f"qT{p}", name=f"qT{p}")
                for p in range(NP)
            ]
            kTn = [
                persist.tile([P, tq], BF16, tag=f"kTn{p}", name=f"kTn{p}")
                for p in range(NP)
            ]
            vaug = persist.tile([P, HPC, nkt_tot, DH + 2], BF16, tag="vaug")
            attT = [
                persist.tile([P, tq], BF16, tag=f"attT{p}", name=f"attT{p}")
                for p in range(NP)
            ]
            kTc = [
                kTcp.tile([P, tc], BF16, tag=f"kTc{p}", name=f"kTc{p}")
                for p in range(NP)
            ]

            wq_r = projw.tile([P, nch, gw], BF16, tag="wqr")
            nc.sync.dma_start(wq_r, wq_d.ap().rearrange("(a p) n -> p a n", p=P))
            wkv_r = projw.tile([P, nch, 2 * gw], BF16, tag="wkvr")
            nc.sync.dma_start(wkv_r, wkv_d.ap().rearrange("(a p) n -> p a n", p=P))
            xT = [
                projw.tile([P, tq], BF16, tag=f"xT{c}", name=f"xT{c}")
                for c in range(nch)
            ]
            for c in range(nch):
                nc.sync.dma_start(xT[c], xT_d.ap()[c * P : (c + 1) * P, :])
            # cache/wo loads after the projection-critical inputs
            for p in range(NP):
                nc.sync.dma_start(kTc[p], kcpT_d.ap()[p])
            for h in range(HPC):
                nc.sync.dma_start(
                    vaug[:, h, :nkt_cache, :],
                    vca_d.ap()[h].rearrange("(n p) d -> p n d", p=P),
                )
            nc.sync.dma_start(wo_r, wo_d.ap().rearrange("(a p) n -> p a n", p=P))

            # ---- projection chain emitters (interleaved into attention to
            # keep the PE array dense/warm during exp waits) ----
            def proj_qk_chain(w_t, dst, bias, p, ts):
                pq = avp.tile([P, QT], F32, tag="av", name="pq")
                for c in range(nch):
                    nc.tensor.matmul(
                        pq,
                        w_t[:, c, p * P : (p + 1) * P],
                        xT[c][:, ts * QT : (ts + 1) * QT],
                        start=(c == 0),
                        stop=(c == nch - 1),
                    )
                nc.vector.tensor_scalar_add(
                    dst[p][:, ts * QT : (ts + 1) * QT], pq, bias[:, p : p + 1]
                )

            def proj_kv_chain(t):
                pk = avp.tile([P, 2 * gw], F32, tag="av", name="pk")
                for c in range(nch):
                    nc.tensor.matmul(
                        pk,
                        xT[c][:, t * P : (t + 1) * P],
                        wkv_r[:, c, :],
                        start=(c == 0),
                        stop=(c == nch - 1),
                    )
                kvnat = stage.tile([P, 2 * gw], F32, tag="kvnat")
                nc.vector.tensor_tensor(kvnat, pk, bkvrep_sb, mybir.AluOpType.add)
                nc.gpsimd.dma_start(
                    knew_d.ap()[t * P : (t + 1) * P, :], kvnat[:, :gw]
                )
                nc.gpsimd.dma_start(
                    vnew_d.ap()[t * P : (t + 1) * P, :], kvnat[:, gw:]
                )
                nc.vector.tensor_copy(
                    vaug[:, :, nkt_cache + t, :DH],
                    kvnat[:, gw:].rearrange("p (h d) -> p h d", h=HPC),
                )

            def proj_chains_for_ts(ts):
                chains = []
                for p in range(NP):
                    chains.append(
                        lambda p=p: proj_qk_chain(wq_r, qT, bqt_sb, p, ts)
                    )
                for p in range(NP):
                    chains.append(
                        lambda p=p: proj_qk_chain(wkv_r, kTn, bkt_sb, p, ts)
                    )
                for t in range(ts * (QT // P), (ts + 1) * (QT // P)):
                    chains.append(lambda t=t: proj_kv_chain(t))
                return chains

            def ones_fill():
                # ones column for the new-token ktiles (copy from cache part)
                nc.vector.tensor_copy(
                    vaug[:, :, nkt_cache : nkt_cache + nkt_new, DH : DH + 1],
                    vaug[:, :, 0:nkt_new, DH : DH + 1],
                )

            # proj for the first q chunk runs up front
            for ch in proj_chains_for_ts(0):
                ch()
            ones_fill()

            # ---- attention + interleaved proj + output projection ----
            def oproj_chain(m, ns):
                py = avp.tile([P, QT], F32, tag="av", name="py")
                for p in range(NP):
                    nc.tensor.matmul(
                        py,
                        attT[p][:, m * P : (m + 1) * P],
                        wo_r[:, p, ns * QT : (ns + 1) * QT],
                        start=(p == 0),
                        stop=(p == NP - 1),
                    )
                ysb = stage.tile([P, QT], F32, tag="ysb")
                nc.vector.tensor_copy(ysb, py)
                nc.gpsimd.dma_start(
                    y_d.ap()[m * P : (m + 1) * P, ns * QT : (ns + 1) * QT], ysb
                )

            qoff = tc  # key j visible to query i iff j <= i + tc
            pending = []
            for qt in range(nqt):
                if qt + 1 < nqt:
                    pending.extend(proj_chains_for_ts(qt + 1))
                nk = min((qt * QT + QT - 1 + qoff) // P + 1, nkt_tot)
                for p in range(NP):
                    for h in range(2):
                        hd = slice(h * DH, (h + 1) * DH)
                        av = avp.tile([P, QT], F32, tag="av", name="av")
                        ngrp = (nk + GK - 1) // GK
                        for g in range(ngrp):
                            kts = list(range(g * GK, min((g + 1) * GK, nk)))
                            sc = psD.tile([P, GK * QT], F32, tag="sc")
                            for j, kt in enumerate(kts):
                                kt_src = (
                                    kTc[p][hd, kt * P : (kt + 1) * P]
                                    if kt < nkt_cache
                                    else kTn[p][
                                        hd,
                                        (kt - nkt_cache) * P
                                        : (kt - nkt_cache + 1) * P,
                                    ]
                                )
                                nc.tensor.matmul(
                                    sc[:, j * QT : (j + 1) * QT],
                                    kt_src,
                                    qT[p][hd, qt * QT : (qt + 1) * QT],
                                    start=True,
                                    stop=True,
                                )
                            nexp = len(kts) * QT
                            ex = expp.tile([P, GK * QT], BF16, tag="ex")
                            nc.scalar.activation(
                                ex[:, :nexp],
                                sc[:, :nexp],
                                mybir.ActivationFunctionType.Exp,
                                bias=0.0,
                                scale=SCALE,
                            )
                            for j, kt in enumerate(kts):
                                d = kt * P - (qt * QT + qoff)
                                if d + P - 1 > 0:
                                    sl = ex[:, j * QT : (j + 1) * QT]
                                    nc.gpsimd.affine_select(
                                        out=sl,
                                        in_=sl,
                                        compare_op=mybir.AluOpType.is_ge,
                                        fill=0.0,
                                        base=-d,
                                        channel_multiplier=-1,
                                        pattern=[[1, QT]],
                                    )
                            if g % 2 == 0 and pending:
                                pending.pop(0)()
                            for j, kt in enumerate(kts):
                                nc.tensor.matmul(
                                    av[0 : DH + 1, :],
                                    vaug[:, 2 * p + h, kt, 0 : DH + 1],
                                    ex[:, j * QT : (j + 1) * QT],
                                    start=(kt == 0),
                                    stop=(kt == nk - 1),
                                )
                        # normalize -> attT; copy out of PSUM first so the
                        # av bank frees after one DVE op; head B hops to
                        # partitions 64-127 via sbuf->sbuf DMA
                        avs = normp.tile([P, QT], F32, tag="avs")
                        nc.vector.tensor_copy(avs[0 : DH + 1, :], av[0 : DH + 1, :])
                        row0 = normp.tile([1, QT], F32, tag="row0")
                        nc.sync.dma_start(row0, avs[DH : DH + 1, :])
                        rep = normp.tile([DH, QT], F32, tag="rep")
                        nc.gpsimd.partition_broadcast(rep, row0)
                        rec = normp.tile([DH, QT], F32, tag="rec")
                        nc.vector.reciprocal_approx_fast(out=rec, in_=rep)
                        if h == 0:
                            nc.vector.tensor_tensor(
                                attT[p][0:DH, qt * QT : (qt + 1) * QT],
                                avs[0:DH, :],
                                rec,
                                mybir.AluOpType.mult,
                            )
                        else:
                            tmp = normp.tile([DH, QT], BF16, tag="tmpB")
                            nc.vector.tensor_tensor(
                                tmp, avs[0:DH, :], rec, mybir.AluOpType.mult
                            )
                            nc.sync.dma_start(
                                attT[p][DH:P, qt * QT : (qt + 1) * QT], tmp
                            )
                # this q tile's output projection joins the pending work,
                # consumed across the next tile's head boundaries
                for m in range(qt * (QT // P), (qt + 1) * (QT // P)):
                    for ns in range(D // QT):
                        pending.append(lambda m=m, ns=ns: oproj_chain(m, ns))
            while pending:
                pending.pop(0)()

    nc.compile()
    return nc


_NC_CACHE = {}


def _get_nc(tq=TQ_TOT, tc=TC):
    key = (tq, tc)
    if key not in _NC_CACHE:
        _NC_CACHE[key] = build_nc(tq, tc)
    return _NC_CACHE[key]


def make_in_maps(x, k_cache, v_cache, Wq, bq, Wk, bk, Wv, bv, Wo, bo):
    """Build the 8 per-core input maps from full inputs (host marshalling:
    head-group slicing, transposes to channel-major, bf16 downcasts)."""
    gw = HPC * DH
    in_maps = []
    for core in range(8):
        b = core // 4
        g = core % 4
        hs = slice(4 * g, 4 * g + 4)
        xT = np.ascontiguousarray(x[b].T.astype(NPBF))
        # pair kT cache: [NP, 128, tc], head A on partitions 0-63, B on 64-127
        kcpT = np.ascontiguousarray(
            k_cache[b, hs].transpose(0, 2, 1).reshape(NP, 2 * DH, -1).astype(NPBF)
        )
        vca = np.zeros((HPC, v_cache.shape[2], DH + 2), dtype=NPBF)
        vca[:, :, DH] = 1
        vca[:, :, :DH] = v_cache[b, hs].astype(NPBF)
        bq_s = bq[gw * g : gw * (g + 1)]
        bk_s = bk[gw * g : gw * (g + 1)]
        bv_s = bv[gw * g : gw * (g + 1)]
        in_maps.append(
            {
                "xT": xT,
                "kcpT": kcpT,
                "vca": np.ascontiguousarray(vca),
                "wq": np.ascontiguousarray(
                    Wq[:, gw * g : gw * (g + 1)].astype(NPBF)
                ),
                "wkv": np.ascontiguousarray(
                    np.concatenate(
                        [
                            Wk[:, gw * g : gw * (g + 1)],
                            Wv[:, gw * g : gw * (g + 1)],
                        ],
                        axis=1,
                    ).astype(NPBF)
                ),
                "wo": np.ascontiguousarray(
                    Wo[gw * g : gw * (g + 1), :].astype(NPBF)
                ),
                "bqt": np.ascontiguousarray(bq_s.reshape(NP, P).T),
                "bkt": np.ascontiguousarray(bk_s.reshape(NP, P).T),
                "bkvrep": np.ascontiguousarray(
                    np.broadcast_to(np.concatenate([bk_s, bv_s]), (P, 2 * gw))
                ),
            }
        )
    return in_maps


def assemble(results, k_cache, v_cache, bo, tq=TQ_TOT):
    out = np.empty((B, tq, D), dtype=np.float32)
    tcache = k_cache.shape[2]
    k_full = np.empty((B, H, tcache + tq, DH), dtype=np.float32)
    v_full = np.empty_like(k_full)
    k_full[:, :, :tcache] = k_cache
    v_full[:, :, :tcache] = v_cache
    for b in range(B):
        acc = None
        for g in range(4):
            r = results[b * 4 + g]
            acc = r["y"].copy() if acc is None else acc + r["y"]
            k_full[b, 4 * g : 4 * g + 4, tcache:] = (
                r["knew"].reshape(tq, HPC, DH).transpose(1, 0, 2)
            )
            v_full[b, 4 * g : 4 * g + 4, tcache:] = (
                r["vnew"].reshape(tq, HPC, DH).transpose(1, 0, 2)
            )
        out[b] = acc + bo
    return out, (k_full, v_full)


def kernel(x, k_cache, v_cache, Wq, bq, Wk, bk, Wv, bv, Wo, bo, _run_kw=None):
    args = [
        np.ascontiguousarray(np.asarray(a, dtype=np.float32))
        for a in (x, k_cache, v_cache, Wq, bq, Wk, bk, Wv, bv, Wo, bo)
    ]
    x, k_cache, v_cache, Wq, bq, Wk, bk, Wv, bv, Wo, bo = args
    nc = _get_nc()
    in_maps = make_in_maps(*args)
    res = run_bass_kernel_spmd(
        nc, in_maps, core_ids=list(range(8)), **(_run_kw or {})
    )
    kernel.last_result = res
    return assemble(res.results, k_cache, v_cache, bo)


# revision 16
# speedup vs baseline: 1.0221x; 1.0221x over previous
"""Causal self-attention with KV-cache append on 8 Trainium2 NeuronCores.

Sharding: tensor-parallel over heads (4 groups of 4 heads) x data-parallel
over batch (2): core c handles batch b=c//4 and heads [4*(c%4), 4*(c%4)+4).
Each core computes its QKV projections, causal attention for its heads, and a
partial output projection (row-parallel Wo).  The host sums the 4 partial
outputs per batch, adds bo, and assembles the returned KV caches.  Host-side
input marshalling per core: slice the head group, transpose x and the K cache
to channel-major, downcast the matmul-side tensors to bf16, append a ones
column to the V cache (it rides the A.V matmul to produce the softmax
denominator).

Device kernel notes:
 - projections contract over d_model using the host-transposed xT; all
   matmuls run in bf16 (PSUM accumulates fp32); k_new/v_new outputs are
   computed to fp32 with the bias added on the DVE.
 - attention runs in the "scores transposed" layout: scoresT[k, q] tiles of
   [128, 512] = kT.T @ qT, one head at a time, with the score PSUM
   double-buffered against the scalar engine's exp.
 - softmax skips the max subtraction (scores ~ N(0,1) so exp is safe);
   exp runs on the scalar engine straight out of 3-bank PSUM into bf16 sbuf
   tiles; causal masking is gpsimd.affine_select on the exp tiles.
 - normalization: sumexp row (row 64 of the A.V PSUM) -> partition 0 via an
   sbuf->sbuf DMA -> gpsimd partition_broadcast -> DVE reciprocal_approx_fast
   -> multiply; head B's half hops to partitions 64-127 via sbuf->sbuf DMA.
 - the output projection is interleaved per q-tile so the PE never idles at
   the tail.
"""

import sys
import types

import numpy as np
import ml_dtypes

import concourse.bass as bass
import concourse.mybir as mybir
import concourse.tile as tile
from concourse import bacc
from concourse.bass_utils import run_bass_kernel_spmd


def _ensure_axon_ntff_shim():
    """This image's ``antenv`` lacks ``axon_hooks``; provide the tiny
    get/set pair so ``run_bass_kernel_spmd(trace=True)`` works instead of
    crashing on import (it is only touched when tracing is requested)."""
    try:
        import antenv.axon_hooks  # noqa: F401

        return
    except ImportError:
        pass
    try:
        import antenv

        mod = types.ModuleType("antenv.axon_hooks")
        mod._hook = None
        mod.set_axon_ntff_profile_hook = lambda h: setattr(mod, "_hook", h)
        mod.get_axon_ntff_profile_hook = lambda: mod._hook
        sys.modules["antenv.axon_hooks"] = mod
        antenv.axon_hooks = mod
        try:
            from trn_agent_boot.trn_boot import _ntff_profile_via_ctypes

            mod._hook = _ntff_profile_via_ctypes("/opt/axon/libaxon_pjrt.so")
        except Exception:
            pass
    except Exception:
        pass


_ensure_axon_ntff_shim()

# problem shape (hardcoded per harness contract)
B = 2
D = 1024
H = 16
DH = 64
TQ_TOT = 2048
TC = 2048

HPC = H // 4  # heads per core = 4
NP = HPC // 2  # head pairs per core = 2
P = 128
QT = 512  # q tile width
GK = 3  # k-tiles (of 128) per exp group
SCALE = DH**-0.5

F32 = mybir.dt.float32
BF16 = mybir.dt.bfloat16
NPBF = ml_dtypes.bfloat16


def build_nc(tq=TQ_TOT, tc=TC):
    """Build the per-core SPMD program (same program on all 8 cores)."""
    nqt = tq // QT
    nkt_new = tq // P
    nkt_cache = tc // P
    nkt_tot = nkt_new + nkt_cache
    nxt = tq // P  # token tiles
    nch = D // P  # 8 chan chunks
    gw = HPC * DH  # per-core projection width (256)

    nc = bacc.Bacc("TRN2", target_bir_lowering=False, debug=False, num_devices=8)

    xT_d = nc.dram_tensor("xT", [D, tq], BF16, kind="ExternalInput")
    kcpT_d = nc.dram_tensor("kcpT", [NP, P, tc], BF16, kind="ExternalInput")
    vca_d = nc.dram_tensor("vca", [HPC, tc, DH + 2], BF16, kind="ExternalInput")
    wq_d = nc.dram_tensor("wq", [D, gw], BF16, kind="ExternalInput")
    wkv_d = nc.dram_tensor("wkv", [D, 2 * gw], BF16, kind="ExternalInput")
    wo_d = nc.dram_tensor("wo", [gw, D], BF16, kind="ExternalInput")
    bqt_d = nc.dram_tensor("bqt", [P, NP], F32, kind="ExternalInput")
    bkt_d = nc.dram_tensor("bkt", [P, NP], F32, kind="ExternalInput")
    bkvrep_d = nc.dram_tensor("bkvrep", [P, 2 * gw], F32, kind="ExternalInput")

    y_d = nc.dram_tensor("y", [tq, D], F32, kind="ExternalOutput")
    knew_d = nc.dram_tensor("knew", [tq, gw], F32, kind="ExternalOutput")
    vnew_d = nc.dram_tensor("vnew", [tq, gw], F32, kind="ExternalOutput")

    with tile.TileContext(nc) as tc_:
        with (
            tc_.tile_pool(name="const", bufs=1) as const,
            tc_.tile_pool(name="persist", bufs=1) as persist,
            tc_.tile_pool(name="stage", bufs=2) as stage,
            tc_.tile_pool(name="kTcp", bufs=1) as kTcp,
            tc_.tile_pool(name="projw", bufs=1) as projw,
            tc_.tile_pool(name="psD", bufs=2, space="PSUM") as psD,
            tc_.tile_pool(name="avp", bufs=2, space="PSUM") as avp,
            tc_.tile_pool(name="expp", bufs=4) as expp,
            tc_.tile_pool(name="normp", bufs=2) as normp,
        ):
            # ---- constants / inputs straight to sbuf ----
            bqt_sb = const.tile([P, NP], F32, tag="bqt")
            bkt_sb = const.tile([P, NP], F32, tag="bkt")
            nc.gpsimd.dma_start(bqt_sb, bqt_d.ap())
            nc.gpsimd.dma_start(bkt_sb, bkt_d.ap())
            bkvrep_sb = const.tile([P, 2 * gw], F32, tag="bkvrep")
            nc.gpsimd.dma_start(bkvrep_sb, bkvrep_d.ap())
            wo_r = const.tile([P, gw // P, D], BF16, tag="wor")

            # persistent activation tensors
            qT = [
                persist.tile([P, tq], BF16, tag=f"qT{p}", name=f"qT{p}")
                for p in range(NP)
            ]
            kTn = [
                persist.tile([P, tq], BF16, tag=f"kTn{p}", name=f"kTn{p}")
                for p in range(NP)
            ]
            vaug = persist.tile([P, HPC, nkt_tot, DH + 2], BF16, tag="vaug")
            attT = [
                persist.tile([P, tq], BF16, tag=f"attT{p}", name=f"attT{p}")
                for p in range(NP)
            ]
            kTc = [
                kTcp.tile([P, tc], BF16, tag=f"kTc{p}", name=f"kTc{p}")
                for p in range(NP)
            ]

            wq_r = projw.tile([P, nch, gw], BF16, tag="wqr")
            nc.sync.dma_start(wq_r, wq_d.ap().rearrange("(a p) n -> p a n", p=P))
            wkv_r = projw.tile([P, nch, 2 * gw], BF16, tag="wkvr")
            nc.sync.dma_start(wkv_r, wkv_d.ap().rearrange("(a p) n -> p a n", p=P))
            xT = [
                projw.tile([P, tq], BF16, tag=f"xT{c}", name=f"xT{c}")
                for c in range(nch)
            ]
            for c in range(nch):
                nc.sync.dma_start(xT[c], xT_d.ap()[c * P : (c + 1) * P, :])
            # cache/wo loads after the projection-critical inputs
            for p in range(NP):
                nc.sync.dma_start(kTc[p], kcpT_d.ap()[p])
            for h in range(HPC):
                nc.sync.dma_start(
                    vaug[:, h, :nkt_cache, :],
                    vca_d.ap()[h].rearrange("(n p) d -> p n d", p=P),
                )
            nc.sync.dma_start(wo_r, wo_d.ap().rearrange("(a p) n -> p a n", p=P))

            # ---- projection chain emitters (interleaved into attention to
            # keep the PE array dense/warm during exp waits) ----
            def proj_qk_chain(w_t, dst, bias, p, ts):
                pq = avp.tile([P, QT], F32, tag="av", name="pq")
                for c in range(nch):
                    nc.tensor.matmul(
                        pq,
                        w_t[:, c, p * P : (p + 1) * P],
                        xT[c][:, ts * QT : (ts + 1) * QT],
                        start=(c == 0),
                        stop=(c == nch - 1),
                    )
                nc.vector.tensor_scalar_add(
                    dst[p][:, ts * QT : (ts + 1) * QT], pq, bias[:, p : p + 1]
                )

            def proj_kv_chain(t):
                pk = avp.tile([P, 2 * gw], F32, tag="av", name="pk")
                for c in range(nch):
                    nc.tensor.matmul(
                        pk,
                        xT[c][:, t * P : (t + 1) * P],
                        wkv_r[:, c, :],
                        start=(c == 0),
                        stop=(c == nch - 1),
                    )
                kvnat = stage.tile([P, 2 * gw], F32, tag="kvnat")
                nc.vector.tensor_tensor(kvnat, pk, bkvrep_sb, mybir.AluOpType.add)
                nc.gpsimd.dma_start(
                    knew_d.ap()[t * P : (t + 1) * P, :], kvnat[:, :gw]
                )
                nc.gpsimd.dma_start(
                    vnew_d.ap()[t * P : (t + 1) * P, :], kvnat[:, gw:]
                )
                nc.vector.tensor_copy(
                    vaug[:, :, nkt_cache + t, :DH],
                    kvnat[:, gw:].rearrange("p (h d) -> p h d", h=HPC),
                )

            def proj_chains_for_ts(ts):
                chains = []
                for p in range(NP):
                    chains.append(
                        lambda p=p: proj_qk_chain(wq_r, qT, bqt_sb, p, ts)
                    )
                for p in range(NP):
                    chains.append(
                        lambda p=p: proj_qk_chain(wkv_r, kTn, bkt_sb, p, ts)
                    )
                for t in range(ts * (QT // P), (ts + 1) * (QT // P)):
                    chains.append(lambda t=t: proj_kv_chain(t))
                return chains

            def ones_fill():
                # ones column for the new-token ktiles (copy from cache part)
                nc.vector.tensor_copy(
                    vaug[:, :, nkt_cache : nkt_cache + nkt_new, DH : DH + 1],
                    vaug[:, :, 0:nkt_new, DH : DH + 1],
                )

            # proj for the first q chunk runs up front
            for ch in proj_chains_for_ts(0):
                ch()
            ones_fill()

            # ---- attention + interleaved proj + output projection ----
            def oproj_chain(m, ns):
                py = avp.tile([P, QT], F32, tag="av", name="py")
                for p in range(NP):
                    nc.tensor.matmul(
                        py,
                        attT[p][:, m * P : (m + 1) * P],
                        wo_r[:, p, ns * QT : (ns + 1) * QT],
                        start=(p == 0),
                        stop=(p == NP - 1),
                    )
                ysb = stage.tile([P, QT], F32, tag="ysb")
                nc.vector.tensor_copy(ysb, py)
                nc.gpsimd.dma_start(
                    y_d.ap()[m * P : (m + 1) * P, ns * QT : (ns + 1) * QT], ysb
                )

            qoff = tc  # key j visible to query i iff j <= i + tc
            pending = []
            for qt in range(nqt):
                if qt + 1 < nqt:
                    pending.extend(proj_chains_for_ts(qt + 1))
                nk = min((qt * QT + QT - 1 + qoff) // P + 1, nkt_tot)
                for p in range(NP):
                    for h in range(2):
                        hd = slice(h * DH, (h + 1) * DH)
                        av = avp.tile([P, QT], F32, tag="av", name="av")
                        ngrp = (nk + GK - 1) // GK
                        for g in range(ngrp):
                            kts = list(range(g * GK, min((g + 1) * GK, nk)))
                            sc = psD.tile([P, GK * QT], F32, tag="sc")
                            for j, kt in enumerate(kts):
                                kt_src = (
                                    kTc[p][hd, kt * P : (kt + 1) * P]
                                    if kt < nkt_cache
                                    else kTn[p][
                                        hd,
                                        (kt - nkt_cache) * P
                                        : (kt - nkt_cache + 1) * P,
                                    ]
                                )
                                nc.tensor.matmul(
                                    sc[:, j * QT : (j + 1) * QT],
                                    kt_src,
                                    qT[p][hd, qt * QT : (qt + 1) * QT],
                                    start=True,
                                    stop=True,
                                )
                            nexp = len(kts) * QT
                            ex = expp.tile([P, GK * QT], BF16, tag="ex")
                            nc.scalar.activation(
                                ex[:, :nexp],
                                sc[:, :nexp],
                                mybir.ActivationFunctionType.Exp,
                                bias=0.0,
                                scale=SCALE,
                            )
                            if g % 2 == 0 and pending:
                                pending.pop(0)()
                            for j, kt in enumerate(kts):
                                d = kt * P - (qt * QT + qoff)
                                if d + P - 1 > 0:
                                    sl = ex[:, j * QT : (j + 1) * QT]
                                    nc.gpsimd.affine_select(
                                        out=sl,
                                        in_=sl,
                                        compare_op=mybir.AluOpType.is_ge,
                                        fill=0.0,
                                        base=-d,
                                        channel_multiplier=-1,
                                        pattern=[[1, QT]],
                                    )
                            for j, kt in enumerate(kts):
                                nc.tensor.matmul(
                                    av[0 : DH + 1, :],
                                    vaug[:, 2 * p + h, kt, 0 : DH + 1],
                                    ex[:, j * QT : (j + 1) * QT],
                                    start=(kt == 0),
                                    stop=(kt == nk - 1),
                                )
                        # normalize -> attT; copy out of PSUM first so the
                        # av bank frees after one DVE op; head B hops to
                        # partitions 64-127 via sbuf->sbuf DMA
                        avs = normp.tile([P, QT], F32, tag="avs")
                        nc.vector.tensor_copy(avs[0 : DH + 1, :], av[0 : DH + 1, :])
                        row0 = normp.tile([1, QT], F32, tag="row0")
                        nc.sync.dma_start(row0, avs[DH : DH + 1, :])
                        rep = normp.tile([DH, QT], F32, tag="rep")
                        nc.gpsimd.partition_broadcast(rep, row0)
                        rec = normp.tile([DH, QT], F32, tag="rec")
                        nc.vector.reciprocal_approx_fast(out=rec, in_=rep)
                        if h == 0:
                            nc.vector.tensor_tensor(
                                attT[p][0:DH, qt * QT : (qt + 1) * QT],
                                avs[0:DH, :],
                                rec,
                                mybir.AluOpType.mult,
                            )
                        else:
                            tmp = normp.tile([DH, QT], BF16, tag="tmpB")
                            nc.vector.tensor_tensor(
                                tmp, avs[0:DH, :], rec, mybir.AluOpType.mult
                            )
                            nc.sync.dma_start(
                                attT[p][DH:P, qt * QT : (qt + 1) * QT], tmp
                            )
                # this q tile's output projection joins the pending work,
                # consumed across the next tile's head boundaries
                for m in range(qt * (QT // P), (qt + 1) * (QT // P)):
                    for ns in range(D // QT):
                        pending.append(lambda m=m, ns=ns: oproj_chain(m, ns))
            while pending:
                pending.pop(0)()

    nc.compile()
    return nc


_NC_CACHE = {}


def _get_nc(tq=TQ_TOT, tc=TC):
    key = (tq, tc)
    if key not in _NC_CACHE:
        _NC_CACHE[key] = build_nc(tq, tc)
    return _NC_CACHE[key]


def make_in_maps(x, k_cache, v_cache, Wq, bq, Wk, bk, Wv, bv, Wo, bo):
    """Build the 8 per-core input maps from full inputs (host marshalling:
    head-group slicing, transposes to channel-major, bf16 downcasts)."""
    gw = HPC * DH
    in_maps = []
    for core in range(8):
        b = core // 4
        g = core % 4
        hs = slice(4 * g, 4 * g + 4)
        xT = np.ascontiguousarray(x[b].T.astype(NPBF))
        # pair kT cache: [NP, 128, tc], head A on partitions 0-63, B on 64-127
        kcpT = np.ascontiguousarray(
            k_cache[b, hs].transpose(0, 2, 1).reshape(NP, 2 * DH, -1).astype(NPBF)
        )
        vca = np.zeros((HPC, v_cache.shape[2], DH + 2), dtype=NPBF)
        vca[:, :, DH] = 1
        vca[:, :, :DH] = v_cache[b, hs].astype(NPBF)
        bq_s = bq[gw * g : gw * (g + 1)]
        bk_s = bk[gw * g : gw * (g + 1)]
        bv_s = bv[gw * g : gw * (g + 1)]
        in_maps.append(
            {
                "xT": xT,
                "kcpT": kcpT,
                "vca": np.ascontiguousarray(vca),
                "wq": np.ascontiguousarray(
                    Wq[:, gw * g : gw * (g + 1)].astype(NPBF)
                ),
                "wkv": np.ascontiguousarray(
                    np.concatenate(
                        [
                            Wk[:, gw * g : gw * (g + 1)],
                            Wv[:, gw * g : gw * (g + 1)],
                        ],
                        axis=1,
                    ).astype(NPBF)
                ),
                "wo": np.ascontiguousarray(
                    Wo[gw * g : gw * (g + 1), :].astype(NPBF)
                ),
                "bqt": np.ascontiguousarray(bq_s.reshape(NP, P).T),
                "bkt": np.ascontiguousarray(bk_s.reshape(NP, P).T),
                "bkvrep": np.ascontiguousarray(
                    np.broadcast_to(np.concatenate([bk_s, bv_s]), (P, 2 * gw))
                ),
            }
        )
    return in_maps


def assemble(results, k_cache, v_cache, bo, tq=TQ_TOT):
    out = np.empty((B, tq, D), dtype=np.float32)
    tcache = k_cache.shape[2]
    k_full = np.empty((B, H, tcache + tq, DH), dtype=np.float32)
    v_full = np.empty_like(k_full)
    k_full[:, :, :tcache] = k_cache
    v_full[:, :, :tcache] = v_cache
    for b in range(B):
        acc = None
        for g in range(4):
            r = results[b * 4 + g]
            acc = r["y"].copy() if acc is None else acc + r["y"]
            k_full[b, 4 * g : 4 * g + 4, tcache:] = (
                r["knew"].reshape(tq, HPC, DH).transpose(1, 0, 2)
            )
            v_full[b, 4 * g : 4 * g + 4, tcache:] = (
                r["vnew"].reshape(tq, HPC, DH).transpose(1, 0, 2)
            )
        out[b] = acc + bo
    return out, (k_full, v_full)


def kernel(x, k_cache, v_cache, Wq, bq, Wk, bk, Wv, bv, Wo, bo, _run_kw=None):
    args = [
        np.ascontiguousarray(np.asarray(a, dtype=np.float32))
        for a in (x, k_cache, v_cache, Wq, bq, Wk, bk, Wv, bv, Wo, bo)
    ]
    x, k_cache, v_cache, Wq, bq, Wk, bk, Wv, bv, Wo, bo = args
    nc = _get_nc()
    in_maps = make_in_maps(*args)
    res = run_bass_kernel_spmd(
        nc, in_maps, core_ids=list(range(8)), **(_run_kw or {})
    )
    kernel.last_result = res
    return assemble(res.results, k_cache, v_cache, bo)


# revision 17
# speedup vs baseline: 1.0507x; 1.0279x over previous
"""Causal self-attention with KV-cache append on 8 Trainium2 NeuronCores.

Sharding: tensor-parallel over heads (4 groups of 4 heads) x data-parallel
over batch (2): core c handles batch b=c//4 and heads [4*(c%4), 4*(c%4)+4).
Each core computes its QKV projections, causal attention for its heads, and a
partial output projection (row-parallel Wo).  The host sums the 4 partial
outputs per batch, adds bo, and assembles the returned KV caches.  Host-side
input marshalling per core: slice the head group, transpose x and the K cache
to channel-major, downcast the matmul-side tensors to bf16, append a ones
column to the V cache (it rides the A.V matmul to produce the softmax
denominator).

Device kernel notes:
 - projections contract over d_model using the host-transposed xT; all
   matmuls run in bf16 (PSUM accumulates fp32); k_new/v_new outputs are
   computed to fp32 with the bias added on the DVE.
 - attention runs in the "scores transposed" layout: scoresT[k, q] tiles of
   [128, 512] = kT.T @ qT, one head at a time, with the score PSUM
   double-buffered against the scalar engine's exp.
 - softmax skips the max subtraction (scores ~ N(0,1) so exp is safe);
   exp runs on the scalar engine straight out of 3-bank PSUM into bf16 sbuf
   tiles; causal masking is gpsimd.affine_select on the exp tiles.
 - normalization: sumexp row (row 64 of the A.V PSUM) -> partition 0 via an
   sbuf->sbuf DMA -> gpsimd partition_broadcast -> DVE reciprocal_approx_fast
   -> multiply; head B's half hops to partitions 64-127 via sbuf->sbuf DMA.
 - the output projection is interleaved per q-tile so the PE never idles at
   the tail.
"""

import sys
import types

import numpy as np
import ml_dtypes

import concourse.bass as bass
import concourse.mybir as mybir
import concourse.tile as tile
from concourse import bacc
from concourse.bass_utils import run_bass_kernel_spmd


def _ensure_axon_ntff_shim():
    """This image's ``antenv`` lacks ``axon_hooks``; provide the tiny
    get/set pair so ``run_bass_kernel_spmd(trace=True)`` works instead of
    crashing on import (it is only touched when tracing is requested)."""
    try:
        import antenv.axon_hooks  # noqa: F401

        return
    except ImportError:
        pass
    try:
        import antenv

        mod = types.ModuleType("antenv.axon_hooks")
        mod._hook = None
        mod.set_axon_ntff_profile_hook = lambda h: setattr(mod, "_hook", h)
        mod.get_axon_ntff_profile_hook = lambda: mod._hook
        sys.modules["antenv.axon_hooks"] = mod
        antenv.axon_hooks = mod
        try:
            from trn_agent_boot.trn_boot import _ntff_profile_via_ctypes

            mod._hook = _ntff_profile_via_ctypes("/opt/axon/libaxon_pjrt.so")
        except Exception:
            pass
    except Exception:
        pass


_ensure_axon_ntff_shim()

# problem shape (hardcoded per harness contract)
B = 2
D = 1024
H = 16
DH = 64
TQ_TOT = 2048
TC = 2048

HPC = H // 4  # heads per core = 4
NP = HPC // 2  # head pairs per core = 2
P = 128
QT = 512  # q tile width
GK = 3  # k-tiles (of 128) per exp group
SCALE = DH**-0.5

F32 = mybir.dt.float32
BF16 = mybir.dt.bfloat16
NPBF = ml_dtypes.bfloat16


def build_nc(tq=TQ_TOT, tc=TC):
    """Build the per-core SPMD program (same program on all 8 cores)."""
    nqt = tq // QT
    nkt_new = tq // P
    nkt_cache = tc // P
    nkt_tot = nkt_new + nkt_cache
    nxt = tq // P  # token tiles
    nch = D // P  # 8 chan chunks
    gw = HPC * DH  # per-core projection width (256)

    nc = bacc.Bacc("TRN2", target_bir_lowering=False, debug=False, num_devices=8)

    xT_d = nc.dram_tensor("xT", [D, tq], BF16, kind="ExternalInput")
    kcpT_d = nc.dram_tensor("kcpT", [NP, P, tc], BF16, kind="ExternalInput")
    vca_d = nc.dram_tensor("vca", [HPC, tc, DH + 2], BF16, kind="ExternalInput")
    wq_d = nc.dram_tensor("wq", [D, gw], BF16, kind="ExternalInput")
    wkv_d = nc.dram_tensor("wkv", [D, 2 * gw], BF16, kind="ExternalInput")
    wo_d = nc.dram_tensor("wo", [gw, D], BF16, kind="ExternalInput")
    bqt_d = nc.dram_tensor("bqt", [P, NP], F32, kind="ExternalInput")
    bkt_d = nc.dram_tensor("bkt", [P, NP], F32, kind="ExternalInput")
    bkvrep_d = nc.dram_tensor("bkvrep", [P, 2 * gw], F32, kind="ExternalInput")

    y_d = nc.dram_tensor("y", [tq, D], F32, kind="ExternalOutput")
    knew_d = nc.dram_tensor("knew", [tq, gw], F32, kind="ExternalOutput")
    vnew_d = nc.dram_tensor("vnew", [tq, gw], F32, kind="ExternalOutput")

    with tile.TileContext(nc) as tc_:
        with (
            tc_.tile_pool(name="const", bufs=1) as const,
            tc_.tile_pool(name="persist", bufs=1) as persist,
            tc_.tile_pool(name="stage", bufs=2) as stage,
            tc_.tile_pool(name="kTcp", bufs=1) as kTcp,
            tc_.tile_pool(name="projw", bufs=1) as projw,
            tc_.tile_pool(name="psD", bufs=2, space="PSUM") as psD,
            tc_.tile_pool(name="avp", bufs=2, space="PSUM") as avp,
            tc_.tile_pool(name="expp", bufs=4) as expp,
            tc_.tile_pool(name="normp", bufs=2) as normp,
        ):
            # ---- constants / inputs straight to sbuf ----
            bqt_sb = const.tile([P, NP], F32, tag="bqt")
            bkt_sb = const.tile([P, NP], F32, tag="bkt")
            nc.gpsimd.dma_start(bqt_sb, bqt_d.ap())
            nc.gpsimd.dma_start(bkt_sb, bkt_d.ap())
            bkvrep_sb = const.tile([P, 2 * gw], F32, tag="bkvrep")
            nc.gpsimd.dma_start(bkvrep_sb, bkvrep_d.ap())
            wo_r = const.tile([P, gw // P, D], BF16, tag="wor")

            # persistent activation tensors
            qT = [
                persist.tile([P, tq], BF16, tag=f"qT{p}", name=f"qT{p}")
                for p in range(NP)
            ]
            kTn = [
                persist.tile([P, tq], BF16, tag=f"kTn{p}", name=f"kTn{p}")
                for p in range(NP)
            ]
            vaug = persist.tile([P, HPC, nkt_tot, DH + 2], BF16, tag="vaug")
            attT = [
                persist.tile([P, tq], BF16, tag=f"attT{p}", name=f"attT{p}")
                for p in range(NP)
            ]
            kTc = [
                kTcp.tile([P, tc], BF16, tag=f"kTc{p}", name=f"kTc{p}")
                for p in range(NP)
            ]

            wq_r = projw.tile([P, nch, gw], BF16, tag="wqr")
            nc.sync.dma_start(wq_r, wq_d.ap().rearrange("(a p) n -> p a n", p=P))
            wkv_r = projw.tile([P, nch, 2 * gw], BF16, tag="wkvr")
            nc.sync.dma_start(wkv_r, wkv_d.ap().rearrange("(a p) n -> p a n", p=P))
            xT = [
                projw.tile([P, tq], BF16, tag=f"xT{c}", name=f"xT{c}")
                for c in range(nch)
            ]
            for c in range(nch):
                nc.sync.dma_start(xT[c], xT_d.ap()[c * P : (c + 1) * P, :])
            # cache/wo loads after the projection-critical inputs
            for p in range(NP):
                nc.sync.dma_start(kTc[p], kcpT_d.ap()[p])
            for h in range(HPC):
                nc.sync.dma_start(
                    vaug[:, h, :nkt_cache, :],
                    vca_d.ap()[h].rearrange("(n p) d -> p n d", p=P),
                )
            nc.sync.dma_start(wo_r, wo_d.ap().rearrange("(a p) n -> p a n", p=P))

            # ---- projection chain emitters (interleaved into attention to
            # keep the PE array dense/warm during exp waits) ----
            def proj_qk_chain(w_t, dst, bias, p, ts):
                pq = avp.tile([P, QT], F32, tag="av", name="pq")
                for c in range(nch):
                    nc.tensor.matmul(
                        pq,
                        w_t[:, c, p * P : (p + 1) * P],
                        xT[c][:, ts * QT : (ts + 1) * QT],
                        start=(c == 0),
                        stop=(c == nch - 1),
                    )
                nc.vector.tensor_scalar_add(
                    dst[p][:, ts * QT : (ts + 1) * QT], pq, bias[:, p : p + 1]
                )

            def proj_kv_chain(t):
                pk = avp.tile([P, 2 * gw], F32, tag="av", name="pk")
                for c in range(nch):
                    nc.tensor.matmul(
                        pk,
                        xT[c][:, t * P : (t + 1) * P],
                        wkv_r[:, c, :],
                        start=(c == 0),
                        stop=(c == nch - 1),
                    )
                kvnat = stage.tile([P, 2 * gw], F32, tag="kvnat")
                nc.vector.tensor_tensor(kvnat, pk, bkvrep_sb, mybir.AluOpType.add)
                nc.gpsimd.dma_start(
                    knew_d.ap()[t * P : (t + 1) * P, :], kvnat[:, :gw]
                )
                nc.gpsimd.dma_start(
                    vnew_d.ap()[t * P : (t + 1) * P, :], kvnat[:, gw:]
                )
                nc.vector.tensor_copy(
                    vaug[:, :, nkt_cache + t, :DH],
                    kvnat[:, gw:].rearrange("p (h d) -> p h d", h=HPC),
                )

            def proj_chains_for_ts(ts):
                chains = []
                for p in range(NP):
                    chains.append(
                        lambda p=p: proj_qk_chain(wq_r, qT, bqt_sb, p, ts)
                    )
                for p in range(NP):
                    chains.append(
                        lambda p=p: proj_qk_chain(wkv_r, kTn, bkt_sb, p, ts)
                    )
                for t in range(ts * (QT // P), (ts + 1) * (QT // P)):
                    chains.append(lambda t=t: proj_kv_chain(t))
                return chains

            def ones_fill():
                # ones column for the new-token ktiles (copy from cache part)
                nc.vector.tensor_copy(
                    vaug[:, :, nkt_cache : nkt_cache + nkt_new, DH : DH + 1],
                    vaug[:, :, 0:nkt_new, DH : DH + 1],
                )

            # proj for the first q chunk runs up front
            for ch in proj_chains_for_ts(0):
                ch()
            ones_fill()

            # ---- attention + interleaved proj + output projection ----
            def oproj_chain(m, ns):
                py = avp.tile([P, QT], F32, tag="av", name="py")
                for p in range(NP):
                    nc.tensor.matmul(
                        py,
                        attT[p][:, m * P : (m + 1) * P],
                        wo_r[:, p, ns * QT : (ns + 1) * QT],
                        start=(p == 0),
                        stop=(p == NP - 1),
                    )
                ysb = stage.tile([P, QT], F32, tag="ysb")
                nc.vector.tensor_copy(ysb, py)
                nc.gpsimd.dma_start(
                    y_d.ap()[m * P : (m + 1) * P, ns * QT : (ns + 1) * QT], ysb
                )

            qoff = tc  # key j visible to query i iff j <= i + tc
            pending = []
            for qt in range(nqt):
                if qt + 1 < nqt:
                    pending.extend(proj_chains_for_ts(qt + 1))
                nk = min((qt * QT + QT - 1 + qoff) // P + 1, nkt_tot)
                for p in range(NP):
                    for h in range(2):
                        hd = slice(h * DH, (h + 1) * DH)
                        av = avp.tile([P, QT], F32, tag="av", name="av")
                        ngrp = (nk + GK - 1) // GK
                        for g in range(ngrp):
                            kts = list(range(g * GK, min((g + 1) * GK, nk)))
                            sc = psD.tile([P, GK * QT], F32, tag="sc")
                            for j, kt in enumerate(kts):
                                kt_src = (
                                    kTc[p][hd, kt * P : (kt + 1) * P]
                                    if kt < nkt_cache
                                    else kTn[p][
                                        hd,
                                        (kt - nkt_cache) * P
                                        : (kt - nkt_cache + 1) * P,
                                    ]
                                )
                                nc.tensor.matmul(
                                    sc[:, j * QT : (j + 1) * QT],
                                    kt_src,
                                    qT[p][hd, qt * QT : (qt + 1) * QT],
                                    start=True,
                                    stop=True,
                                )
                            nexp = len(kts) * QT
                            ex = expp.tile([P, GK * QT], BF16, tag="ex")
                            nc.scalar.activation(
                                ex[:, :nexp],
                                sc[:, :nexp],
                                mybir.ActivationFunctionType.Exp,
                                bias=0.0,
                                scale=SCALE,
                            )
                            for j, kt in enumerate(kts):
                                d = kt * P - (qt * QT + qoff)
                                if d + P - 1 > 0:
                                    sl = ex[:, j * QT : (j + 1) * QT]
                                    nc.gpsimd.affine_select(
                                        out=sl,
                                        in_=sl,
                                        compare_op=mybir.AluOpType.is_ge,
                                        fill=0.0,
                                        base=-d,
                                        channel_multiplier=-1,
                                        pattern=[[1, QT]],
                                    )
                            for j, kt in enumerate(kts):
                                nc.tensor.matmul(
                                    av[0 : DH + 1, :],
                                    vaug[:, 2 * p + h, kt, 0 : DH + 1],
                                    ex[:, j * QT : (j + 1) * QT],
                                    start=(kt == 0),
                                    stop=(kt == nk - 1),
                                )
                        # normalize -> attT; copy out of PSUM first so the
                        # av bank frees after one DVE op; head B hops to
                        # partitions 64-127 via sbuf->sbuf DMA
                        avs = normp.tile([P, QT], F32, tag="avs")
                        nc.vector.tensor_copy(avs[0 : DH + 1, :], av[0 : DH + 1, :])
                        row0 = normp.tile([1, QT], F32, tag="row0")
                        nc.sync.dma_start(row0, avs[DH : DH + 1, :])
                        rep = normp.tile([DH, QT], F32, tag="rep")
                        nc.gpsimd.partition_broadcast(rep, row0)
                        rec = normp.tile([DH, QT], F32, tag="rec")
                        nc.vector.reciprocal_approx_fast(out=rec, in_=rep)
                        if h == 0:
                            nc.vector.tensor_tensor(
                                attT[p][0:DH, qt * QT : (qt + 1) * QT],
                                avs[0:DH, :],
                                rec,
                                mybir.AluOpType.mult,
                            )
                        else:
                            tmp = normp.tile([DH, QT], BF16, tag="tmpB")
                            nc.vector.tensor_tensor(
                                tmp, avs[0:DH, :], rec, mybir.AluOpType.mult
                            )
                            nc.sync.dma_start(
                                attT[p][DH:P, qt * QT : (qt + 1) * QT], tmp
                            )
                        # a few independent PE chains (next-chunk proj /
                        # previous-tile oproj) keep the array full while ACT
                        # works through this head's exp
                        for _ in range(4):
                            if pending:
                                pending.pop(0)()
                # this q tile's output projection joins the pending work,
                # consumed across the next tile's head boundaries
                for m in range(qt * (QT // P), (qt + 1) * (QT // P)):
                    for ns in range(D // QT):
                        pending.append(lambda m=m, ns=ns: oproj_chain(m, ns))
            while pending:
                pending.pop(0)()

    nc.compile()
    return nc


_NC_CACHE = {}


def _get_nc(tq=TQ_TOT, tc=TC):
    key = (tq, tc)
    if key not in _NC_CACHE:
        _NC_CACHE[key] = build_nc(tq, tc)
    return _NC_CACHE[key]


def make_in_maps(x, k_cache, v_cache, Wq, bq, Wk, bk, Wv, bv, Wo, bo):
    """Build the 8 per-core input maps from full inputs (host marshalling:
    head-group slicing, transposes to channel-major, bf16 downcasts)."""
    gw = HPC * DH
    in_maps = []
    for core in range(8):
        b = core // 4
        g = core % 4
        hs = slice(4 * g, 4 * g + 4)
        xT = np.ascontiguousarray(x[b].T.astype(NPBF))
        # pair kT cache: [NP, 128, tc], head A on partitions 0-63, B on 64-127
        kcpT = np.ascontiguousarray(
            k_cache[b, hs].transpose(0, 2, 1).reshape(NP, 2 * DH, -1).astype(NPBF)
        )
        vca = np.zeros((HPC, v_cache.shape[2], DH + 2), dtype=NPBF)
        vca[:, :, DH] = 1
        vca[:, :, :DH] = v_cache[b, hs].astype(NPBF)
        bq_s = bq[gw * g : gw * (g + 1)]
        bk_s = bk[gw * g : gw * (g + 1)]
        bv_s = bv[gw * g : gw * (g + 1)]
        in_maps.append(
            {
                "xT": xT,
                "kcpT": kcpT,
                "vca": np.ascontiguousarray(vca),
                "wq": np.ascontiguousarray(
                    Wq[:, gw * g : gw * (g + 1)].astype(NPBF)
                ),
                "wkv": np.ascontiguousarray(
                    np.concatenate(
                        [
                            Wk[:, gw * g : gw * (g + 1)],
                            Wv[:, gw * g : gw * (g + 1)],
                        ],
                        axis=1,
                    ).astype(NPBF)
                ),
                "wo": np.ascontiguousarray(
                    Wo[gw * g : gw * (g + 1), :].astype(NPBF)
                ),
                "bqt": np.ascontiguousarray(bq_s.reshape(NP, P).T),
                "bkt": np.ascontiguousarray(bk_s.reshape(NP, P).T),
                "bkvrep": np.ascontiguousarray(
                    np.broadcast_to(np.concatenate([bk_s, bv_s]), (P, 2 * gw))
                ),
            }
        )
    return in_maps


def assemble(results, k_cache, v_cache, bo, tq=TQ_TOT):
    out = np.empty((B, tq, D), dtype=np.float32)
    tcache = k_cache.shape[2]
    k_full = np.empty((B, H, tcache + tq, DH), dtype=np.float32)
    v_full = np.empty_like(k_full)
    k_full[:, :, :tcache] = k_cache
    v_full[:, :, :tcache] = v_cache
    for b in range(B):
        acc = None
        for g in range(4):
            r = results[b * 4 + g]
            acc = r["y"].copy() if acc is None else acc + r["y"]
            k_full[b, 4 * g : 4 * g + 4, tcache:] = (
                r["knew"].reshape(tq, HPC, DH).transpose(1, 0, 2)
            )
            v_full[b, 4 * g : 4 * g + 4, tcache:] = (
                r["vnew"].reshape(tq, HPC, DH).transpose(1, 0, 2)
            )
        out[b] = acc + bo
    return out, (k_full, v_full)


def kernel(x, k_cache, v_cache, Wq, bq, Wk, bk, Wv, bv, Wo, bo, _run_kw=None):
    args = [
        np.ascontiguousarray(np.asarray(a, dtype=np.float32))
        for a in (x, k_cache, v_cache, Wq, bq, Wk, bk, Wv, bv, Wo, bo)
    ]
    x, k_cache, v_cache, Wq, bq, Wk, bk, Wv, bv, Wo, bo = args
    nc = _get_nc()
    in_maps = make_in_maps(*args)
    res = run_bass_kernel_spmd(
        nc, in_maps, core_ids=list(range(8)), **(_run_kw or {})
    )
    kernel.last_result = res
    return assemble(res.results, k_cache, v_cache, bo)
